# revision 7
# baseline (speedup 1.0000x reference)
"""Trainium2 Bass kernel for nn_MemoryUnit (scatter_memory).

Computation (per row n of x [N, F], memory W [M, F], temperature T):
    logits = x @ W.T                      # [N, M]
    a      = softmax(logits / T, axis=1)
    shrink: out = relu(a-l)*a / (|a-l|+e) (l=0.0025, e=1e-6)
    att    = out / (sum(out) + 1e-8)      (rows with sum<1e-8 -> uniform 1/M)
    output = att @ W                      # [N, F]
    returns (output, att)

Kernel algebra (s = exp(logits/T) per row, S = sum_m s, no max-subtraction --
logits/T is bounded ~[-9, 9] for randn inputs so exp is safe in fp32):
    a = s/S;  for a>l: out = a*(a-l)/((a-l)+e), else 0.
    Let v = S*out = s*u/(u+eS) with u = relu(s-lS).
    Identity:  v = u*(u+lS)/(u+eS)  (exact in both branches)
             = (d - eS) + (l-e)S - (l-e)*e*S^2 / d,   d = u+eS = max(s-(l-e)S, eS)
    =>  v0 = d - B*rec,  rec = 1/d,  B = (l-e)*e*S^2,  v = v0 + (l-2e)S
    att = c*v + b,  c = 1/(V + 1e-8*S),  V = sum_m v = sum(v0) + M*(l-2e)S
    (zero_mask rows: c=0, b=1/M)
    output = att @ W  (via PE transpose of att + matmul)

Sharding: data-parallel over rows across 8 cores; W replicated; no collectives.
"""

import os
import sys

import numpy as np

for _p in ("/opt/trn_rl_repo", "/opt/pypackages"):
    if os.path.isdir(_p) and _p not in sys.path:
        sys.path.insert(0, _p)

from contextlib import ExitStack

import concourse.bacc as bacc
import concourse.bass as bass
import concourse.tile as tile
from concourse import mybir
from concourse.bass_utils import run_bass_kernel_spmd

MEM_DIM = 2000
FEA = 256
N_FULL = 65536
N_CORES = 8
LAM = 0.0025
EPS = 1e-6
M_PAD = 2048  # 16 blocks of 128 (2000 rounded up)
M_BLOCKS = M_PAD // 128

f32 = mybir.dt.float32


def build_nc(rows: int, inv_temp: float) -> bass.Bass:
    AF = mybir.ActivationFunctionType
    OP = mybir.AluOpType

    nc = bacc.Bacc(None)
    n_tiles = rows // 128

    xT = nc.declare_dram_parameter("xT", [FEA, rows], f32, isOutput=False)
    wT = nc.declare_dram_parameter("wT", [FEA, MEM_DIM], f32, isOutput=False)
    wp = nc.declare_dram_parameter("wp", [M_PAD, FEA], f32, isOutput=False)
    ident = nc.declare_dram_parameter("ident", [128, 128], f32, isOutput=False)
    out_d = nc.declare_dram_parameter("out", [rows, FEA], f32, isOutput=True)
    att_d = nc.declare_dram_parameter("att", [rows, MEM_DIM], f32, isOutput=True)

    # mm1 free-dim chunks (fp32 moving operand max 512)
    chunks = [(0, 512), (512, 1024), (1024, 1536), (1536, MEM_DIM)]

    with ExitStack() as ctx:
        tc = ctx.enter_context(tile.TileContext(nc))
        consts = ctx.enter_context(tc.tile_pool(name="consts", bufs=1))
        p_x = ctx.enter_context(tc.tile_pool(name="p_x", bufs=3))
        p_s = ctx.enter_context(tc.tile_pool(name="p_s", bufs=2))
        p_d = ctx.enter_context(tc.tile_pool(name="p_d", bufs=2))
        p_rec = ctx.enter_context(tc.tile_pool(name="p_rec", bufs=2))
        p_v = ctx.enter_context(tc.tile_pool(name="p_v", bufs=2))
        p_att = ctx.enter_context(tc.tile_pool(name="p_att", bufs=2))
        p_attT = ctx.enter_context(tc.tile_pool(name="p_attT", bufs=2))
        p_out = ctx.enter_context(tc.tile_pool(name="p_out", bufs=3))
        p_sm = ctx.enter_context(tc.tile_pool(name="p_sm", bufs=3))
        ps_l = ctx.enter_context(tc.tile_pool(name="ps_l", bufs=1, space="PSUM"))
        ps_t = ctx.enter_context(tc.tile_pool(name="ps_t", bufs=1, space="PSUM"))
        ps_o = ctx.enter_context(tc.tile_pool(name="ps_o", bufs=2, space="PSUM"))

        # ---- constants ----
        wT_sb = consts.tile([128, 2, MEM_DIM], f32)
        nc.sync.dma_start(wT_sb, wT.rearrange("(k p) m -> p k m", p=128))
        w_sb = consts.tile([128, M_BLOCKS, FEA], f32)
        nc.sync.dma_start(w_sb, wp.rearrange("(b p) f -> p b f", p=128))
        id_sb = consts.tile([128, 128], f32)
        nc.sync.dma_start(id_sb, ident[:, :])

        xT_r = xT.rearrange("(k p) n -> p k n", p=128)

        for i in range(n_tiles):
            n0 = i * 128
            # -- load xT tile: [128, 2, 128] (partition = feature % 128) --
            xt = p_x.tile([128, 2, 128], f32)
            nc.sync.dma_start(xt, xT_r[:, :, n0 : n0 + 128])

            # -- mm1: logits[n, m] = sum_f x[n,f] W[m,f] --
            ps_logits = ps_l.tile([128, MEM_DIM], f32)
            for k in range(2):
                for c0, c1 in chunks:
                    nc.tensor.matmul(
                        ps_logits[:, c0:c1],
                        lhsT=xt[:, k, :],
                        rhs=wT_sb[:, k, c0:c1],
                        start=(k == 0),
                        stop=(k == 1),
                    )

            # -- s = exp(logits/T), S = rowsum(s) --
            s = p_s.tile([128, MEM_DIM], f32)
            S = p_sm.tile([128, 1], f32)
            nc.scalar.activation(s, ps_logits, AF.Exp, scale=inv_temp, accum_out=S)

            # -- per-row scalars (stage 1) --
            sc_sub = p_sm.tile([128, 1], f32)  # (l-e)*S
            nc.vector.tensor_scalar(sc_sub, S, LAM - EPS, None, op0=OP.mult)
            sc_eps = p_sm.tile([128, 1], f32)  # e*S
            nc.vector.tensor_scalar(sc_eps, S, EPS, None, op0=OP.mult)
            negB = p_sm.tile([128, 1], f32)  # -(l-e)*e*S^2
            nc.vector.scalar_tensor_tensor(
                negB, in0=S, scalar=-(LAM - EPS) * EPS, in1=S, op0=OP.mult, op1=OP.mult
            )
            Sk2 = p_sm.tile([128, 1], f32)  # (l-2e)*S
            nc.vector.tensor_scalar(Sk2, S, LAM - 2 * EPS, None, op0=OP.mult)

            # -- d = max(s - (l-e)S, eS) --
            d = p_d.tile([128, MEM_DIM], f32)
            nc.vector.tensor_scalar(d, s, sc_sub, sc_eps, op0=OP.subtract, op1=OP.max)

            # -- rec ~= 1/d (51 ULP) --
            rec = p_rec.tile([128, MEM_DIM], f32)
            nc.vector.reciprocal_approx_fast(rec, d)

            # -- v0 = d - B*rec ; V0 = rowsum(v0) --
            v0 = p_v.tile([128, MEM_DIM], f32)
            V0 = p_sm.tile([128, 1], f32)
            nc.vector.scalar_tensor_tensor(
                v0, in0=rec, scalar=negB, in1=d, op0=OP.mult, op1=OP.add, accum_out=V0
            )

            # -- per-row scalars (stage 2): c and b0 --
            V = p_sm.tile([128, 1], f32)  # V = V0 + M*(l-2e)*S
            nc.vector.scalar_tensor_tensor(
                V, in0=Sk2, scalar=float(MEM_DIM), in1=V0, op0=OP.mult, op1=OP.add
            )
            denom = p_sm.tile([128, 1], f32)  # V + 1e-8*S
            nc.vector.scalar_tensor_tensor(
                denom, in0=S, scalar=1e-8, in1=V, op0=OP.mult, op1=OP.add
            )
            cden = p_sm.tile([128, 1], f32)
            nc.vector.reciprocal(cden, denom)
            tmask = p_sm.tile([128, 1], f32)  # V - 1e-8*S
            nc.vector.scalar_tensor_tensor(
                tmask, in0=S, scalar=-1e-8, in1=V, op0=OP.mult, op1=OP.add
            )
            mask = p_sm.tile([128, 1], f32)  # 1.0 where degenerate row
            nc.vector.tensor_scalar(mask, tmask, 0.0, None, op0=OP.is_lt)
            notm = p_sm.tile([128, 1], f32)
            nc.vector.tensor_scalar(notm, mask, -1.0, 1.0, op0=OP.mult, op1=OP.add)
            c_sc = p_sm.tile([128, 1], f32)
            nc.vector.tensor_tensor(c_sc, notm, cden, op=OP.mult)
            b_sc = p_sm.tile([128, 1], f32)  # mask / M
            nc.vector.tensor_scalar(b_sc, mask, 1.0 / MEM_DIM, None, op0=OP.mult)
            b0 = p_sm.tile([128, 1], f32)  # c*(l-2e)S + b
            nc.vector.scalar_tensor_tensor(
                b0, in0=Sk2, scalar=c_sc, in1=b_sc, op0=OP.mult, op1=OP.add
            )

            # -- att = c*v0 + b0 (ACT identity with per-partition scale/bias) --
            att_t = p_att.tile([128, M_PAD], f32)
            nc.scalar.activation(
                att_t[:, :MEM_DIM], v0, AF.Identity, bias=b0, scale=c_sc
            )
            nc.gpsimd.memset(att_t[:, MEM_DIM:], 0.0)
            nc.sync.dma_start(att_d[n0 : n0 + 128, :], att_t[:, :MEM_DIM])

            # -- transpose att (PE) + copyback (ACT), 2 half-chunks of 8 blocks --
            attT = p_attT.tile([128, M_PAD], f32)
            for half in range(2):
                ps_tr = ps_t.tile([128, 1024], f32)
                for b8 in range(8):
                    b = half * 8 + b8
                    nc.tensor.transpose(
                        ps_tr[:, b8 * 128 : (b8 + 1) * 128],
                        att_t[:, b * 128 : (b + 1) * 128],
                        id_sb,
                    )
                nc.scalar.activation(
                    attT[:, half * 1024 : (half + 1) * 1024], ps_tr, AF.Copy
                )

            # -- mm2: out[n, f] = sum_m att[n, m] W[m, f] --
            ps_out = ps_o.tile([128, FEA], f32)
            for b in range(M_BLOCKS):
                nc.tensor.matmul(
                    ps_out,
                    lhsT=attT[:, b * 128 : (b + 1) * 128],
                    rhs=w_sb[:, b, :],
                    start=(b == 0),
                    stop=(b == M_BLOCKS - 1),
                )
            out_t = p_out.tile([128, FEA], f32)
            nc.vector.tensor_copy(out_t, ps_out)
            nc.sync.dma_start(out_d[n0 : n0 + 128, :], out_t)

    nc.finalize()
    return nc


_BUILD_CACHE: dict = {}


def _get_nc(rows: int, inv_temp: float) -> bass.Bass:
    key = (rows, inv_temp)
    if key not in _BUILD_CACHE:
        _BUILD_CACHE[key] = build_nc(rows, inv_temp)
    return _BUILD_CACHE[key]


def make_in_maps(x: np.ndarray, w: np.ndarray, n_cores: int = N_CORES):
    n = x.shape[0]
    rows = n // n_cores
    xT_full = np.ascontiguousarray(x.T.astype(np.float32))  # [F, N]
    wT = np.ascontiguousarray(w.T.astype(np.float32))  # [F, M]
    wp = np.zeros((M_PAD, FEA), dtype=np.float32)
    wp[:MEM_DIM] = w
    ident = np.eye(128, dtype=np.float32)
    in_maps = []
    for c in range(n_cores):
        in_maps.append(
            {
                "xT": np.ascontiguousarray(xT_full[:, c * rows : (c + 1) * rows]),
                "wT": wT,
                "wp": wp,
                "ident": ident,
            }
        )
    return in_maps, rows


def run_full(input, weight, temperature, trace=False):
    x = np.asarray(input, dtype=np.float32)
    w = np.asarray(weight, dtype=np.float32)
    inv_temp = float(1.0 / np.asarray(temperature, dtype=np.float32))

    in_maps, rows = make_in_maps(x, w)
    nc = _get_nc(rows, inv_temp)
    res = run_bass_kernel_spmd(
        nc, in_maps, core_ids=list(range(N_CORES)), trace=trace
    )
    out = np.concatenate([res.results[c]["out"] for c in range(N_CORES)], axis=0)
    att = np.concatenate([res.results[c]["att"] for c in range(N_CORES)], axis=0)
    return (out, att), res


def kernel(input, weight, temperature):
    (out, att), _ = run_full(input, weight, temperature, trace=False)
    return out, att


# revision 14
# speedup vs baseline: 2.4914x; 2.4914x over previous
"""Trainium2 Bass kernel for nn_MemoryUnit (scatter_memory).

Computation (per row n of x [N, F], memory W [M, F], temperature T):
    logits = x @ W.T                      # [N, M]
    a      = softmax(logits / T, axis=1)
    shrink: out = relu(a-l)*a / (|a-l|+e) (l=0.0025, e=1e-6)
    att    = out / (sum(out) + 1e-8)      (rows with sum<1e-8 -> uniform 1/M)
    output = att @ W                      # [N, F]
    returns (output, att)

Kernel algebra (s = exp(logits/T) per row, S = sum_m s, no max-subtraction --
logits/T is bounded ~[-9, 9] for randn inputs so exp is safe in fp32):
    a = s/S;  for a>l: out = a*(a-l)/((a-l)+e), else 0.
    Let v = S*out = s*u/(u+eS) with u = relu(s-lS).
    Identity:  v = u*(u+lS)/(u+eS)  (exact in both branches)
             = (d - eS) + (l-e)S - (l-e)*e*S^2 / d,   d = u+eS = max(s-(l-e)S, eS)
    =>  v0 = d - B*rec,  rec = 1/d,  B = (l-e)*e*S^2,  v = v0 + (l-2e)S
    att = c*v + b,  c = 1/(V + 1e-8*S),  V = sum_m v = sum(v0) + M*(l-2e)S
    (zero_mask rows: c=0, b=1/M)
    output = att @ W  (via PE transpose of att + matmul)

Sharding: data-parallel over rows across 8 cores; W replicated; no collectives.
"""

import os
import sys

import numpy as np

for _p in ("/opt/trn_rl_repo", "/opt/pypackages"):
    if os.path.isdir(_p) and _p not in sys.path:
        sys.path.insert(0, _p)

from contextlib import ExitStack

import concourse.bacc as bacc
import concourse.bass as bass
import concourse.tile as tile
from concourse import mybir
from concourse.bass_utils import run_bass_kernel_spmd

MEM_DIM = 2000
FEA = 256
N_FULL = 65536
N_CORES = 8
LAM = 0.0025
EPS = 1e-6
M_PAD = 2048  # 16 blocks of 128 (2000 rounded up)
M_BLOCKS = M_PAD // 128

f32 = mybir.dt.float32
f32r = mybir.dt.float32r  # fp32 storage, relaxed-precision matmul at 4x rate


def build_nc(rows: int, inv_temp: float) -> bass.Bass:
    AF = mybir.ActivationFunctionType
    OP = mybir.AluOpType

    nc = bacc.Bacc(None)
    n_tiles = rows // 128

    xT = nc.declare_dram_parameter("xT", [FEA, rows], f32r, isOutput=False)
    wT = nc.declare_dram_parameter("wT", [FEA, MEM_DIM], f32r, isOutput=False)
    wp = nc.declare_dram_parameter("wp", [M_PAD, FEA], f32r, isOutput=False)
    ident = nc.declare_dram_parameter("ident", [128, 128], f32r, isOutput=False)
    out_d = nc.declare_dram_parameter("out", [rows, FEA], f32, isOutput=True)
    att_d = nc.declare_dram_parameter("att", [rows, MEM_DIM], f32, isOutput=True)

    # mm1 free-dim chunks (fp32 moving operand max 512)
    chunks = [(0, 512), (512, 1024), (1024, 1536), (1536, MEM_DIM)]

    with ExitStack() as ctx:
        tc = ctx.enter_context(tile.TileContext(nc))
        consts = ctx.enter_context(tc.tile_pool(name="consts", bufs=1))
        p_x = ctx.enter_context(tc.tile_pool(name="p_x", bufs=3))
        p_s = ctx.enter_context(tc.tile_pool(name="p_s", bufs=2))
        p_d = ctx.enter_context(tc.tile_pool(name="p_d", bufs=2))
        p_rec = ctx.enter_context(tc.tile_pool(name="p_rec", bufs=2))
        p_v = ctx.enter_context(tc.tile_pool(name="p_v", bufs=2))
        p_att = ctx.enter_context(tc.tile_pool(name="p_att", bufs=2))
        p_attT = ctx.enter_context(tc.tile_pool(name="p_attT", bufs=2))
        p_out = ctx.enter_context(tc.tile_pool(name="p_out", bufs=3))
        p_sm = ctx.enter_context(tc.tile_pool(name="p_sm", bufs=3))
        ps_l = ctx.enter_context(tc.tile_pool(name="ps_l", bufs=1, space="PSUM"))
        ps_t = ctx.enter_context(tc.tile_pool(name="ps_t", bufs=1, space="PSUM"))
        ps_o = ctx.enter_context(tc.tile_pool(name="ps_o", bufs=2, space="PSUM"))

        # ---- constants ----
        wT_sb = consts.tile([128, 2, MEM_DIM], f32r)
        nc.sync.dma_start(wT_sb, wT.rearrange("(k p) m -> p k m", p=128))
        w_sb = consts.tile([128, M_BLOCKS, FEA], f32r)
        nc.sync.dma_start(w_sb, wp.rearrange("(b p) f -> p b f", p=128))
        id_sb = consts.tile([128, 128], f32r)
        nc.sync.dma_start(id_sb, ident[:, :])

        xT_r = xT.rearrange("(k p) n -> p k n", p=128)

        for i in range(n_tiles):
            n0 = i * 128
            # -- load xT tile: [128, 2, 128] (partition = feature % 128) --
            xt = p_x.tile([128, 2, 128], f32r)
            nc.sync.dma_start(xt, xT_r[:, :, n0 : n0 + 128])

            # -- mm1: logits[n, m] = sum_f x[n,f] W[m,f] --
            ps_logits = ps_l.tile([128, MEM_DIM], f32)
            for k in range(2):
                for c0, c1 in chunks:
                    nc.tensor.matmul(
                        ps_logits[:, c0:c1],
                        lhsT=xt[:, k, :],
                        rhs=wT_sb[:, k, c0:c1],
                        start=(k == 0),
                        stop=(k == 1),
                    )

            # -- s = exp(logits/T), S = rowsum(s) --
            s = p_s.tile([128, MEM_DIM], f32)
            S = p_sm.tile([128, 1], f32)
            nc.scalar.activation(s, ps_logits, AF.Exp, scale=inv_temp, accum_out=S)

            # -- per-row scalars (stage 1) --
            sc_sub = p_sm.tile([128, 1], f32)  # (l-e)*S
            nc.vector.tensor_scalar(sc_sub, S, LAM - EPS, None, op0=OP.mult)
            sc_eps = p_sm.tile([128, 1], f32)  # e*S
            nc.vector.tensor_scalar(sc_eps, S, EPS, None, op0=OP.mult)
            negB = p_sm.tile([128, 1], f32)  # -(l-e)*e*S^2
            nc.vector.scalar_tensor_tensor(
                negB, in0=S, scalar=-(LAM - EPS) * EPS, in1=S, op0=OP.mult, op1=OP.mult
            )
            Sk2 = p_sm.tile([128, 1], f32)  # (l-2e)*S
            nc.vector.tensor_scalar(Sk2, S, LAM - 2 * EPS, None, op0=OP.mult)

            # -- d = max(s - (l-e)S, eS) --
            d = p_d.tile([128, MEM_DIM], f32)
            nc.vector.tensor_scalar(d, s, sc_sub, sc_eps, op0=OP.subtract, op1=OP.max)

            # -- rec ~= 1/d (51 ULP) --
            rec = p_rec.tile([128, MEM_DIM], f32)
            nc.vector.reciprocal_approx_fast(rec, d)

            # -- v0 = d - B*rec ; V0 = rowsum(v0) --
            v0 = p_v.tile([128, MEM_DIM], f32)
            V0 = p_sm.tile([128, 1], f32)
            nc.vector.scalar_tensor_tensor(
                v0, in0=rec, scalar=negB, in1=d, op0=OP.mult, op1=OP.add, accum_out=V0
            )

            # -- per-row scalars (stage 2): c and b0 --
            V = p_sm.tile([128, 1], f32)  # V = V0 + M*(l-2e)*S
            nc.vector.scalar_tensor_tensor(
                V, in0=Sk2, scalar=float(MEM_DIM), in1=V0, op0=OP.mult, op1=OP.add
            )
            denom = p_sm.tile([128, 1], f32)  # V + 1e-8*S
            nc.vector.scalar_tensor_tensor(
                denom, in0=S, scalar=1e-8, in1=V, op0=OP.mult, op1=OP.add
            )
            cden = p_sm.tile([128, 1], f32)
            nc.vector.reciprocal(cden, denom)
            tmask = p_sm.tile([128, 1], f32)  # V - 1e-8*S
            nc.vector.scalar_tensor_tensor(
                tmask, in0=S, scalar=-1e-8, in1=V, op0=OP.mult, op1=OP.add
            )
            mask = p_sm.tile([128, 1], f32)  # 1.0 where degenerate row
            nc.vector.tensor_scalar(mask, tmask, 0.0, None, op0=OP.is_lt)
            notm = p_sm.tile([128, 1], f32)
            nc.vector.tensor_scalar(notm, mask, -1.0, 1.0, op0=OP.mult, op1=OP.add)
            c_sc = p_sm.tile([128, 1], f32)
            nc.vector.tensor_tensor(c_sc, notm, cden, op=OP.mult)
            b_sc = p_sm.tile([128, 1], f32)  # mask / M
            nc.vector.tensor_scalar(b_sc, mask, 1.0 / MEM_DIM, None, op0=OP.mult)
            b0 = p_sm.tile([128, 1], f32)  # c*(l-2e)S + b
            nc.vector.scalar_tensor_tensor(
                b0, in0=Sk2, scalar=c_sc, in1=b_sc, op0=OP.mult, op1=OP.add
            )

            # -- att = c*v0 + b0 (ACT identity with per-partition scale/bias) --
            att_t = p_att.tile([128, M_PAD], f32r)
            nc.scalar.activation(
                att_t[:, :MEM_DIM], v0, AF.Identity, bias=b0, scale=c_sc
            )
            nc.gpsimd.memset(att_t[:, MEM_DIM:].bitcast(f32), 0.0)
            nc.sync.dma_start(att_d[n0 : n0 + 128, :], att_t[:, :MEM_DIM].bitcast(f32))

            # -- transpose att (PE) + copyback (ACT), 2 half-chunks of 8 blocks --
            attT = p_attT.tile([128, M_PAD], f32r)
            for half in range(2):
                ps_tr = ps_t.tile([128, 1024], f32r)
                for b8 in range(8):
                    b = half * 8 + b8
                    nc.tensor.transpose(
                        ps_tr[:, b8 * 128 : (b8 + 1) * 128],
                        att_t[:, b * 128 : (b + 1) * 128],
                        id_sb,
                    )
                nc.scalar.activation(
                    attT[:, half * 1024 : (half + 1) * 1024], ps_tr, AF.Copy
                )

            # -- mm2: out[n, f] = sum_m att[n, m] W[m, f] --
            ps_out = ps_o.tile([128, FEA], f32)
            for b in range(M_BLOCKS):
                nc.tensor.matmul(
                    ps_out,
                    lhsT=attT[:, b * 128 : (b + 1) * 128],
                    rhs=w_sb[:, b, :],
                    start=(b == 0),
                    stop=(b == M_BLOCKS - 1),
                )
            out_t = p_out.tile([128, FEA], f32)
            nc.vector.tensor_copy(out_t, ps_out)
            nc.sync.dma_start(out_d[n0 : n0 + 128, :], out_t)

    nc.finalize()
    return nc


_BUILD_CACHE: dict = {}


def _get_nc(rows: int, inv_temp: float) -> bass.Bass:
    key = (rows, inv_temp)
    if key not in _BUILD_CACHE:
        _BUILD_CACHE[key] = build_nc(rows, inv_temp)
    return _BUILD_CACHE[key]


def make_in_maps(x: np.ndarray, w: np.ndarray, n_cores: int = N_CORES):
    n = x.shape[0]
    rows = n // n_cores
    xT_full = np.ascontiguousarray(x.T.astype(np.float32))  # [F, N]
    wT = np.ascontiguousarray(w.T.astype(np.float32))  # [F, M]
    wp = np.zeros((M_PAD, FEA), dtype=np.float32)
    wp[:MEM_DIM] = w
    ident = np.eye(128, dtype=np.float32)
    in_maps = []
    for c in range(n_cores):
        in_maps.append(
            {
                "xT": np.ascontiguousarray(xT_full[:, c * rows : (c + 1) * rows]),
                "wT": wT,
                "wp": wp,
                "ident": ident,
            }
        )
    return in_maps, rows


def run_full(input, weight, temperature, trace=False):
    x = np.asarray(input, dtype=np.float32)
    w = np.asarray(weight, dtype=np.float32)
    inv_temp = float(1.0 / np.asarray(temperature, dtype=np.float32))

    in_maps, rows = make_in_maps(x, w)
    nc = _get_nc(rows, inv_temp)
    res = run_bass_kernel_spmd(
        nc, in_maps, core_ids=list(range(N_CORES)), trace=trace
    )
    out = np.concatenate([res.results[c]["out"] for c in range(N_CORES)], axis=0)
    att = np.concatenate([res.results[c]["att"] for c in range(N_CORES)], axis=0)
    return (out, att), res


def kernel(input, weight, temperature):
    (out, att), _ = run_full(input, weight, temperature, trace=False)
    return out, att
